# revision 27
# baseline (speedup 1.0000x reference)
"""Trainium2 Bass kernel for 3-layer GCN (nn_MultiLayerGCN_48773648613817).

Strategy (8 NeuronCores, SPMD):
  - Nodes sharded across cores (12500/core, padded to 12544 = 98*128).
  - Per layer: hsb = dis (.) (X @ W) computed shard-local, AllGather'd into a
    replicated bf16 DRAM table (100352 rows).
  - Edges partitioned by destination core, grouped into 128-node dest windows.
    Within each window, edges are sorted into 4 source-segment classes
    (segment = src_row // 25088, so the int16 gather index fits); each
    (window, class) run is padded to a multiple of 128 (shared layout across
    cores via max-over-cores counts).
  - Gathers use batched dma_gather (InstDMAGatherAnt): K=16 chunks (2048 rows
    of 256B) per instruction, one SWDGE queue per class (num_swdge_queues=4),
    single_packet=False (>64-descriptor packets hang the SDMA engine).
  - Gathers are prefetched msg_bufs batches ahead per class so the DMA
    pipeline overlaps the matmul/S-build pipeline.
  - Per chunk: one-hot S built on DVE via is_equal (SB chunks per op),
    TensorE matmul S^T @ msg accumulated into the window's PSUM; the
    self-loop term rides PSUM via an identity matmul against hsb.
  - Window epilogue (ACT engine): t2 = dis * psum, PE-transpose, then
    xt = relu(t2^T + b) — relu/+b commute with the transpose, which makes b
    a per-partition scalar. Layer 2 keeps [node, feat] orientation and adds
    b via a replicated tile before the output DMA.
"""

import numpy as np

from concourse import bass, bacc, mybir, tile, library_config
from concourse.bass_utils import run_bass_kernel_spmd

N_NODES = 100000
N_LAYERS = 3
DIM = 128
N_CORES = 8
NSH = N_NODES // N_CORES          # 12500 real nodes per shard
P = 128
NWIN = 98                          # windows per shard
NSHP = NWIN * P                    # 12544 padded nodes per shard
N_TABLE = N_CORES * NSHP           # 100352 padded table rows
NSEG = 4
SEGR = N_TABLE // NSEG             # 25088 rows per gather segment

K = 16                             # chunks per dma_gather
SB = 16                            # chunks per S-build op

F32 = mybir.dt.float32
BF = mybir.dt.bfloat16
I16 = mybir.dt.int16


def _prepare(x, edge_indices, W, b):
    """Host-side preprocessing. Returns (in_maps, layout)."""
    x = np.asarray(x, dtype=np.float32)
    ei = np.asarray(edge_indices).astype(np.int64)
    W = np.asarray(W, dtype=np.float32)
    b = np.asarray(b, dtype=np.float32)

    import ml_dtypes
    BF16 = ml_dtypes.bfloat16

    iota_row = np.broadcast_to(
        np.arange(P, dtype=np.float32)[None, :], (P, P)
    ).astype(BF16)
    ident_bf = np.eye(P, dtype=np.float32).astype(BF16)
    bb = b.reshape(1, N_LAYERS * DIM).copy()

    xts = []
    for c in range(N_CORES):
        xp = np.zeros((NSHP, DIM), dtype=np.float32)
        xp[:NSH] = x[c * NSH : (c + 1) * NSH]
        xts.append(np.ascontiguousarray(xp.T))               # [128, 12544]

    degs = np.ones((N_CORES, N_LAYERS, P, NWIN), dtype=np.float32)
    # per (core, layer): edge arrays sorted by (window, class)
    per_core = [[None] * N_LAYERS for _ in range(N_CORES)]
    cnts = np.zeros((N_CORES, N_LAYERS, NWIN, NSEG), dtype=np.int64)

    for l in range(N_LAYERS):
        row = ei[l, 0]
        col = ei[l, 1]
        deg = np.bincount(col, minlength=N_NODES).astype(np.float32) + 1.0
        src_pad = ((row // NSH) * NSHP + (row % NSH)).astype(np.int64)
        seg = (src_pad // SEGR).astype(np.int64)
        idx16 = (src_pad % SEGR).astype(np.int16)
        core_of = col // NSH
        lcol = (col % NSH).astype(np.int64)
        win = lcol // P
        dloc = (lcol % P).astype(np.float32)
        for c in range(N_CORES):
            m = core_of == c
            wc, sc, ic, dc = win[m], seg[m], idx16[m], dloc[m]
            key = wc * NSEG + sc
            order = np.argsort(key, kind="stable")
            wc, sc, ic, dc = wc[order], sc[order], ic[order], dc[order]
            cnt = np.bincount(
                wc * NSEG + sc, minlength=NWIN * NSEG
            ).reshape(NWIN, NSEG)
            per_core[c][l] = (cnt, ic, dc)
            cnts[c, l] = cnt
            dlp = np.ones(NSHP, dtype=np.float32)
            dlp[:NSH] = deg[c * NSH : (c + 1) * NSH]
            degs[c, l] = dlp.reshape(NWIN, P).T

    # shared layout: chunks per (layer, window, class)
    mx = cnts.max(axis=0)                                    # [L, NWIN, NSEG]
    m_chunks = (mx + P - 1) // P                             # chunk counts
    tl_layer = m_chunks.sum(axis=(1, 2))                     # [L]
    tlc_layer = m_chunks.sum(axis=1)                         # [L, NSEG] chunks/class
    tmax = int(tl_layer.max())

    # per-core streams
    dloc_all = np.full((N_CORES, N_LAYERS, P, tmax), -1.0, dtype=np.float32)
    idx_all = np.zeros((N_CORES, N_LAYERS, P, tmax * 8), dtype=np.int16)
    for l in range(N_LAYERS):
        tl = int(tl_layer[l])
        tlc = tlc_layer[l]
        coff = np.concatenate([[0], np.cumsum(tlc)[:-1]])    # class chunk offsets
        for c in range(N_CORES):
            cnt, ic, dc = per_core[c][l]
            off = np.concatenate([[0], np.cumsum(cnt.ravel())[:-1]]).reshape(
                NWIN, NSEG
            )
            # slot stream (dloc) and class streams (idx)
            d_arr = np.full((tl * P,), -1.0, dtype=np.float32)
            i_arr = [np.zeros((int(tlc[s]) * P,), dtype=np.int16)
                     for s in range(NSEG)]
            cpos = [0] * NSEG
            spos = 0
            for w in range(NWIN):
                for s in range(NSEG):
                    mws = int(m_chunks[l, w, s])
                    if mws == 0:
                        continue
                    nreal = int(cnt[w, s])
                    o0 = int(off[w, s])
                    d_arr[spos : spos + nreal] = dc[o0 : o0 + nreal]
                    i_arr[s][cpos[s] * P : cpos[s] * P + nreal] = (
                        ic[o0 : o0 + nreal]
                    )
                    spos += mws * P
                    cpos[s] += mws
            dloc_all[c, l, :, :tl] = d_arr.reshape(tl, P).T
            # wrap idx streams: linear j -> [j%16, j//16], replicated x8
            colpos = 0
            for s in range(NSEG):
                n = i_arr[s].shape[0]
                if n == 0:
                    continue
                wrapped = i_arr[s].reshape(n // 16, 16).T    # [16, n//16]
                idx_all[c, l, :, colpos : colpos + n // 16] = np.tile(
                    wrapped, (8, 1)
                )
                colpos += n // 16

    in_maps = []
    for c in range(N_CORES):
        in_maps.append(
            {
                "xt": xts[c],
                "wmat": W,
                "bb": bb,
                "bcolT": np.ascontiguousarray(b.T),
                "iota": iota_row,
                "identb": ident_bf,
                "degs": degs[c],
                "idxs": idx_all[c],
                "dlocs": dloc_all[c].astype(BF16),
            }
        )
    layout = (m_chunks, tl_layer, tlc_layer, tmax)
    return in_maps, layout


def _build(layout, msg_bufs=4, skip_collective=False, use_act=True, K=K, SB=SB, s_fp8=True, sel_bufs=4):
    m_chunks, tl_layer, tlc_layer, tmax = layout
    nc = bacc.Bacc(
        "TRN2",
        target_bir_lowering=False,
        debug=False,
        num_devices=N_CORES,
        num_swdge_queues=4,
    )
    xt_in = nc.dram_tensor("xt", [P, NSHP], F32, kind="ExternalInput").ap()
    w_in = nc.dram_tensor("wmat", [N_LAYERS, DIM, DIM], F32, kind="ExternalInput").ap()
    b_in = nc.dram_tensor("bb", [1, N_LAYERS * DIM], F32, kind="ExternalInput").ap()
    bcol_in = nc.dram_tensor("bcolT", [DIM, N_LAYERS], F32, kind="ExternalInput").ap()
    iota_in = nc.dram_tensor("iota", [P, P], BF, kind="ExternalInput").ap()
    identb_in = nc.dram_tensor("identb", [P, P], BF, kind="ExternalInput").ap()
    deg_in = nc.dram_tensor("degs", [N_LAYERS, P, NWIN], F32, kind="ExternalInput").ap()
    idxs_in = nc.dram_tensor("idxs", [N_LAYERS, P, tmax * 8], I16, kind="ExternalInput").ap()
    dloc_in = nc.dram_tensor("dlocs", [N_LAYERS, P, tmax], BF, kind="ExternalInput").ap()
    out_ap = nc.dram_tensor("out", [NSHP, DIM], F32, kind="ExternalOutput").ap()

    SDT = mybir.dt.float8e4 if s_fp8 else BF
    hloc16 = nc.dram_tensor("hloc16", [NSHP, DIM], BF).ap()
    table16 = nc.dram_tensor("table16", [N_TABLE, DIM], BF, addr_space="Shared").ap()

    with tile.TileContext(nc) as tc:
        with (
            tc.tile_pool(name="const", bufs=1) as constp,
            tc.tile_pool(name="xt", bufs=1) as xtp,
            tc.tile_pool(name="edges", bufs=1) as edgep,
            tc.tile_pool(name="dl", bufs=2) as dlp,
            tc.tile_pool(name="msg", bufs=msg_bufs) as msgp,
            tc.tile_pool(name="sel", bufs=sel_bufs) as selp,
            tc.tile_pool(name="hsb", bufs=1) as hsbp,
            tc.tile_pool(name="tr", bufs=3) as trp,
            tc.tile_pool(name="ph", bufs=2, space="PSUM") as php,
            tc.tile_pool(name="pw", bufs=2, space="PSUM") as pwp,
            tc.tile_pool(name="pt", bufs=2, space="PSUM") as ptp,
            tc.tile_pool(name="pb", bufs=1, space="PSUM") as pbp,
        ):
            nc.gpsimd.load_library(library_config.mlp)

            iota_sb = constp.tile([P, P], BF)
            nc.sync.dma_start(out=iota_sb[:], in_=iota_in[:])
            identb_sb = constp.tile([P, P], BF)
            nc.sync.dma_start(out=identb_sb[:], in_=identb_in[:])
            w_sb = constp.tile([P, N_LAYERS * DIM], F32)
            for l in range(N_LAYERS):
                nc.sync.dma_start(out=w_sb[:, l * DIM : (l + 1) * DIM], in_=w_in[l])
            brow_sb = constp.tile([1, N_LAYERS * DIM], F32)
            nc.sync.dma_start(out=brow_sb[:], in_=b_in[:])
            bcol_sb = constp.tile([DIM, N_LAYERS], F32)
            nc.sync.dma_start(out=bcol_sb[:], in_=bcol_in[:])
            ones_row = constp.tile([1, P], F32)
            nc.vector.memset(ones_row[:], 1.0)

            xt_sb = xtp.tile([P, NSHP], F32)
            nc.sync.dma_start(out=xt_sb[:], in_=xt_in[:])

            for l in range(N_LAYERS):
                tl = int(tl_layer[l])
                tlc = [int(t) for t in tlc_layer[l]]
                coff = [0] * NSEG
                for s in range(1, NSEG):
                    coff[s] = coff[s - 1] + tlc[s - 1]

                # ---- normalization: dis = 1/sqrt(deg), [dest_p, w] ---------
                deg_sb = trp.tile([P, NWIN], F32, tag="deg")
                nc.sync.dma_start(out=deg_sb[:], in_=deg_in[l])
                dis_sb = trp.tile([P, NWIN], F32, tag="dis")
                nc.vector.reciprocal(dis_sb[:], deg_sb[:])
                nc.scalar.activation(
                    dis_sb[:], dis_sb[:], mybir.ActivationFunctionType.Sqrt
                )

                if l == N_LAYERS - 1:
                    # b replicated per partition (for the untransposed output)
                    pb2 = pbp.tile([P, P], F32, space="PSUM", tag="pb")
                    nc.tensor.matmul(
                        out=pb2[:],
                        lhsT=ones_row[:],
                        rhs=brow_sb[:, l * DIM : (l + 1) * DIM],
                        start=True,
                        stop=True,
                    )
                    bbc_sb = trp.tile([P, P], F32, tag="bbc")
                    nc.vector.tensor_copy(out=bbc_sb[:], in_=pb2[:])

                # ---- edge metadata ----------------------------------------
                idxs_sb = edgep.tile([P, tmax * 8], I16, tag="idxs")
                nc.sync.dma_start(out=idxs_sb[:, : tl * 8], in_=idxs_in[l, :, : tl * 8])
                dloc_sb = dlp.tile([P, tmax], BF, tag="dlocs")
                nc.sync.dma_start(out=dloc_sb[:, :tl], in_=dloc_in[l, :, :tl])

                # ---- H stage: hsb = dis (.) (X @ W), AllGather ------------
                hsb = hsbp.tile([P, NWIN * P], BF, tag="hsb")
                for w in range(NWIN):
                    ph = php.tile([P, P], F32, space="PSUM", tag="ph")
                    nc.tensor.matmul(
                        out=ph[:],
                        lhsT=xt_sb[:, w * P : (w + 1) * P],
                        rhs=w_sb[:, l * DIM : (l + 1) * DIM],
                        start=True,
                        stop=True,
                    )
                    if use_act:
                        nc.scalar.activation(
                            out=hsb[:, w * P : (w + 1) * P],
                            in_=ph[:],
                            func=mybir.ActivationFunctionType.Copy,
                            scale=dis_sb[:, w : w + 1],
                        )
                    else:
                        nc.vector.tensor_scalar(
                            out=hsb[:, w * P : (w + 1) * P],
                            in0=ph[:],
                            scalar1=dis_sb[:, w : w + 1],
                            scalar2=None,
                            op0=mybir.AluOpType.mult,
                        )
                nc.sync.dma_start(
                    out=hloc16[:].rearrange("(w p) f -> p w f", p=P),
                    in_=hsb[:].rearrange("p (w f) -> p w f", f=DIM),
                )
                if skip_collective:
                    nc.sync.dma_start(out=table16[:NSHP, :], in_=hloc16[:])
                else:
                    nc.gpsimd.collective_compute(
                        "AllGather",
                        mybir.AluOpType.bypass,
                        replica_groups=[list(range(N_CORES))],
                        ins=[hloc16[:]],
                        outs=[table16[:]],
                    )

                # ---- scatter stage ----------------------------------------
                cpos = [0] * NSEG
                cur_msg = [None] * NSEG
                s_sb = None
                t = 0
                nbat = [(tlc[s] + K - 1) // K for s in range(NSEG)]
                pending = [[] for _ in range(NSEG)]
                issued = [0] * NSEG

                def issue_gather(s):
                    b = issued[s]
                    nbg = min(K, tlc[s] - b * K)
                    mt = msgp.tile([P, K * P], BF, tag=f"m{s}", name=f"mt{l}_{s}_{b}")
                    nc.gpsimd.dma_gather(
                        mt[:, : nbg * P].rearrange("p (k f) -> p k f", k=nbg),
                        table16[s * SEGR : (s + 1) * SEGR, :],
                        idxs_sb[
                            :,
                            (coff[s] + b * K) * 8 : (coff[s] + b * K) * 8 + nbg * 8,
                        ],
                        nbg * P,
                        nbg * P,
                        P,
                        single_packet=False,
                        queue_num=s,
                    )
                    pending[s].append(mt)
                    issued[s] += 1

                for s in range(NSEG):
                    while issued[s] < min(msg_bufs, nbat[s]):
                        issue_gather(s)
                for w in range(NWIN):
                    wtot = int(m_chunks[l, w].sum())
                    pw = pwp.tile([P, P], F32, space="PSUM", tag="pw")
                    nc.tensor.matmul(
                        out=pw[:],
                        lhsT=identb_sb[:],
                        rhs=hsb[:, w * P : (w + 1) * P],
                        start=True,
                        stop=(wtot == 0),
                    )
                    done = 0
                    for s in range(NSEG):
                        for _ in range(int(m_chunks[l, w, s])):
                            if t % SB == 0:
                                nb = min(SB, tl - t)
                                s_sb = selp.tile([P, SB * P], SDT, tag="sel")
                                nc.vector.tensor_tensor(
                                    out=s_sb[:, : nb * P].rearrange(
                                        "p (k j) -> p k j", k=nb
                                    ),
                                    in0=dloc_sb[:, t : t + nb]
                                    .unsqueeze(2)
                                    .to_broadcast([P, nb, P]),
                                    in1=iota_sb[:]
                                    .unsqueeze(1)
                                    .to_broadcast([P, nb, P]),
                                    op=mybir.AluOpType.is_equal,
                                )
                            cp = cpos[s]
                            if cp % K == 0:
                                cur_msg[s] = pending[s].pop(0)
                                if issued[s] < nbat[s]:
                                    issue_gather(s)
                            done += 1
                            nc.tensor.matmul(
                                out=pw[:],
                                lhsT=s_sb[:, (t % SB) * P : (t % SB + 1) * P],
                                rhs=cur_msg[s][:, (cp % K) * P : (cp % K + 1) * P],
                                start=False,
                                stop=(done == wtot),
                            )
                            cpos[s] += 1
                            t += 1
                    # ---- epilogue -------------------------------------
                    if l < N_LAYERS - 1:
                        # relu and +b commute with the transpose: apply
                        # dis before, b (per-partition post-transpose) after
                        t2 = trp.tile([P, P], BF, tag="t2b")
                        if use_act:
                            nc.scalar.activation(
                                out=t2[:],
                                in_=pw[:],
                                func=mybir.ActivationFunctionType.Copy,
                                scale=dis_sb[:, w : w + 1],
                            )
                        else:
                            nc.vector.tensor_scalar(
                                out=t2[:],
                                in0=pw[:],
                                scalar1=dis_sb[:, w : w + 1],
                                scalar2=None,
                                op0=mybir.AluOpType.mult,
                            )
                        pt = ptp.tile([P, P], BF, space="PSUM", tag="pt")
                        nc.tensor.transpose(
                            out=pt[:], in_=t2[:], identity=identb_sb[:]
                        )
                        if use_act:
                            nc.scalar.activation(
                                out=xt_sb[:, w * P : (w + 1) * P],
                                in_=pt[:],
                                func=mybir.ActivationFunctionType.Relu,
                                bias=bcol_sb[:, l : l + 1],
                            )
                        else:
                            nc.vector.tensor_scalar(
                                out=xt_sb[:, w * P : (w + 1) * P],
                                in0=pt[:],
                                scalar1=bcol_sb[:, l : l + 1],
                                scalar2=0.0,
                                op0=mybir.AluOpType.add,
                                op1=mybir.AluOpType.max,
                            )
                    else:
                        t1 = trp.tile([P, P], F32, tag="t1f")
                        if use_act:
                            nc.scalar.activation(
                                out=t1[:],
                                in_=pw[:],
                                func=mybir.ActivationFunctionType.Copy,
                                scale=dis_sb[:, w : w + 1],
                            )
                        else:
                            nc.vector.tensor_scalar(
                                out=t1[:],
                                in0=pw[:],
                                scalar1=dis_sb[:, w : w + 1],
                                scalar2=None,
                                op0=mybir.AluOpType.mult,
                            )
                        t2f = trp.tile([P, P], F32, tag="t2f")
                        nc.vector.tensor_tensor(
                            out=t2f[:],
                            in0=t1[:],
                            in1=bbc_sb[:],
                            op=mybir.AluOpType.add,
                        )
                        nc.vector.tensor_scalar(
                            out=t2f[:],
                            in0=t2f[:],
                            scalar1=0.0,
                            scalar2=None,
                            op0=mybir.AluOpType.max,
                        )
                        nc.sync.dma_start(
                            out=out_ap[w * P : (w + 1) * P, :], in_=t2f[:]
                        )

    nc.compile()
    return nc


def build_all(x, edge_indices, W, b):
    in_maps, layout = _prepare(x, edge_indices, W, b)
    nc = _build(layout)
    return nc, in_maps


def kernel(x, edge_indices, W, b):
    nc, in_maps = build_all(x, edge_indices, W, b)
    last_err = None
    for _ in range(3):  # retry transient NRT/axon device faults
        try:
            res = run_bass_kernel_spmd(nc, in_maps, list(range(N_CORES)))
            break
        except Exception as e:  # noqa: BLE001
            last_err = e
            import time as _time

            _time.sleep(5.0)
    else:
        raise last_err
    out = np.concatenate(
        [res.results[c]["out"][:NSH] for c in range(N_CORES)], axis=0
    )
    return out.astype(np.float32)


# revision 29
# speedup vs baseline: 1.0427x; 1.0427x over previous
"""Trainium2 Bass kernel for 3-layer GCN (nn_MultiLayerGCN_48773648613817).

Strategy (8 NeuronCores, SPMD):
  - Nodes sharded across cores (12500/core, padded to 12544 = 98*128).
  - Per layer: hsb = dis (.) (X @ W) computed shard-local, AllGather'd into a
    replicated bf16 DRAM table (100352 rows).
  - Edges partitioned by destination core, grouped into 128-node dest windows.
    Within each window, edges are sorted into 4 source-segment classes
    (segment = src_row // 25088, so the int16 gather index fits); each
    (window, class) run is padded to a multiple of 128 (shared layout across
    cores via max-over-cores counts).
  - Gathers use batched dma_gather (InstDMAGatherAnt): K=16 chunks (2048 rows
    of 256B) per instruction, one SWDGE queue per class (num_swdge_queues=4),
    single_packet=False (>64-descriptor packets hang the SDMA engine).
  - Gathers are prefetched msg_bufs batches ahead per class so the DMA
    pipeline overlaps the matmul/S-build pipeline.
  - Per chunk: one-hot S built on DVE via is_equal (SB chunks per op),
    TensorE matmul S^T @ msg accumulated into the window's PSUM; the
    self-loop term rides PSUM via an identity matmul against hsb.
  - Window epilogue (ACT engine): t2 = dis * psum, PE-transpose, then
    xt = relu(t2^T + b) — relu/+b commute with the transpose, which makes b
    a per-partition scalar. Layer 2 keeps [node, feat] orientation and adds
    b via a replicated tile before the output DMA.
"""

import numpy as np

from concourse import bass, bacc, mybir, tile, library_config
from concourse.bass_utils import run_bass_kernel_spmd

N_NODES = 100000
N_LAYERS = 3
DIM = 128
N_CORES = 8
NSH = N_NODES // N_CORES          # 12500 real nodes per shard
P = 128
NWIN = 98                          # windows per shard
NSHP = NWIN * P                    # 12544 padded nodes per shard
N_TABLE = N_CORES * NSHP           # 100352 padded table rows
NSEG = 4
SEGR = N_TABLE // NSEG             # 25088 rows per gather segment

K = 16                             # chunks per dma_gather
SB = 16                            # chunks per S-build op

F32 = mybir.dt.float32
BF = mybir.dt.bfloat16
I16 = mybir.dt.int16


def _prepare(x, edge_indices, W, b):
    """Host-side preprocessing. Returns (in_maps, layout)."""
    x = np.asarray(x, dtype=np.float32)
    ei = np.asarray(edge_indices).astype(np.int64)
    W = np.asarray(W, dtype=np.float32)
    b = np.asarray(b, dtype=np.float32)

    import ml_dtypes
    BF16 = ml_dtypes.bfloat16

    iota_row = np.broadcast_to(
        np.arange(P, dtype=np.float32)[None, :], (P, P)
    ).astype(BF16)
    ident_bf = np.eye(P, dtype=np.float32).astype(BF16)
    bb = b.reshape(1, N_LAYERS * DIM).copy()

    xts = []
    for c in range(N_CORES):
        xp = np.zeros((NSHP, DIM), dtype=np.float32)
        xp[:NSH] = x[c * NSH : (c + 1) * NSH]
        xts.append(np.ascontiguousarray(xp.T))               # [128, 12544]

    degs = np.ones((N_CORES, N_LAYERS, P, NWIN), dtype=np.float32)
    # per (core, layer): edge arrays sorted by (window, class)
    per_core = [[None] * N_LAYERS for _ in range(N_CORES)]
    cnts = np.zeros((N_CORES, N_LAYERS, NWIN, NSEG), dtype=np.int64)

    for l in range(N_LAYERS):
        row = ei[l, 0]
        col = ei[l, 1]
        deg = np.bincount(col, minlength=N_NODES).astype(np.float32) + 1.0
        src_pad = ((row // NSH) * NSHP + (row % NSH)).astype(np.int64)
        seg = (src_pad // SEGR).astype(np.int64)
        idx16 = (src_pad % SEGR).astype(np.int16)
        core_of = col // NSH
        lcol = (col % NSH).astype(np.int64)
        win = lcol // P
        dloc = (lcol % P).astype(np.float32)
        for c in range(N_CORES):
            m = core_of == c
            wc, sc, ic, dc = win[m], seg[m], idx16[m], dloc[m]
            key = wc * NSEG + sc
            order = np.argsort(key, kind="stable")
            wc, sc, ic, dc = wc[order], sc[order], ic[order], dc[order]
            cnt = np.bincount(
                wc * NSEG + sc, minlength=NWIN * NSEG
            ).reshape(NWIN, NSEG)
            per_core[c][l] = (cnt, ic, dc)
            cnts[c, l] = cnt
            dlp = np.ones(NSHP, dtype=np.float32)
            dlp[:NSH] = deg[c * NSH : (c + 1) * NSH]
            degs[c, l] = dlp.reshape(NWIN, P).T

    # shared layout: chunks per (layer, window, class)
    mx = cnts.max(axis=0)                                    # [L, NWIN, NSEG]
    m_chunks = (mx + P - 1) // P                             # chunk counts
    tl_layer = m_chunks.sum(axis=(1, 2))                     # [L]
    tlc_layer = m_chunks.sum(axis=1)                         # [L, NSEG] chunks/class
    tmax = int(tl_layer.max())

    # per-core streams
    dloc_all = np.full((N_CORES, N_LAYERS, P, tmax), -1.0, dtype=np.float32)
    idx_all = np.zeros((N_CORES, N_LAYERS, P, tmax * 8), dtype=np.int16)
    for l in range(N_LAYERS):
        tl = int(tl_layer[l])
        tlc = tlc_layer[l]
        coff = np.concatenate([[0], np.cumsum(tlc)[:-1]])    # class chunk offsets
        for c in range(N_CORES):
            cnt, ic, dc = per_core[c][l]
            off = np.concatenate([[0], np.cumsum(cnt.ravel())[:-1]]).reshape(
                NWIN, NSEG
            )
            # slot stream (dloc) and class streams (idx)
            d_arr = np.full((tl * P,), -1.0, dtype=np.float32)
            i_arr = [np.zeros((int(tlc[s]) * P,), dtype=np.int16)
                     for s in range(NSEG)]
            cpos = [0] * NSEG
            spos = 0
            for w in range(NWIN):
                for s in range(NSEG):
                    mws = int(m_chunks[l, w, s])
                    if mws == 0:
                        continue
                    nreal = int(cnt[w, s])
                    o0 = int(off[w, s])
                    d_arr[spos : spos + nreal] = dc[o0 : o0 + nreal]
                    i_arr[s][cpos[s] * P : cpos[s] * P + nreal] = (
                        ic[o0 : o0 + nreal]
                    )
                    spos += mws * P
                    cpos[s] += mws
            dloc_all[c, l, :, :tl] = d_arr.reshape(tl, P).T
            # wrap idx streams: linear j -> [j%16, j//16], replicated x8
            colpos = 0
            for s in range(NSEG):
                n = i_arr[s].shape[0]
                if n == 0:
                    continue
                wrapped = i_arr[s].reshape(n // 16, 16).T    # [16, n//16]
                idx_all[c, l, :, colpos : colpos + n // 16] = np.tile(
                    wrapped, (8, 1)
                )
                colpos += n // 16

    in_maps = []
    for c in range(N_CORES):
        in_maps.append(
            {
                "xt": xts[c],
                "wmat": W,
                "bb": bb,
                "bcolT": np.ascontiguousarray(b.T),
                "iota": iota_row,
                "identb": ident_bf,
                "degs": degs[c],
                "idxs": idx_all[c],
                "dlocs": dloc_all[c].astype(BF16),
            }
        )
    layout = (m_chunks, tl_layer, tlc_layer, tmax)
    return in_maps, layout


def _build(layout, msg_bufs=4, skip_collective=False, use_act=True, K=K, SB=SB, s_fp8=True, sel_bufs=4, pw_bufs=2, pt_bufs=2, ph_bufs=2):
    m_chunks, tl_layer, tlc_layer, tmax = layout
    nc = bacc.Bacc(
        "TRN2",
        target_bir_lowering=False,
        debug=False,
        num_devices=N_CORES,
        num_swdge_queues=4,
    )
    xt_in = nc.dram_tensor("xt", [P, NSHP], F32, kind="ExternalInput").ap()
    w_in = nc.dram_tensor("wmat", [N_LAYERS, DIM, DIM], F32, kind="ExternalInput").ap()
    b_in = nc.dram_tensor("bb", [1, N_LAYERS * DIM], F32, kind="ExternalInput").ap()
    bcol_in = nc.dram_tensor("bcolT", [DIM, N_LAYERS], F32, kind="ExternalInput").ap()
    iota_in = nc.dram_tensor("iota", [P, P], BF, kind="ExternalInput").ap()
    identb_in = nc.dram_tensor("identb", [P, P], BF, kind="ExternalInput").ap()
    deg_in = nc.dram_tensor("degs", [N_LAYERS, P, NWIN], F32, kind="ExternalInput").ap()
    idxs_in = nc.dram_tensor("idxs", [N_LAYERS, P, tmax * 8], I16, kind="ExternalInput").ap()
    dloc_in = nc.dram_tensor("dlocs", [N_LAYERS, P, tmax], BF, kind="ExternalInput").ap()
    out_ap = nc.dram_tensor("out", [NSHP, DIM], F32, kind="ExternalOutput").ap()

    SDT = mybir.dt.float8e4 if s_fp8 else BF
    hloc16 = nc.dram_tensor("hloc16", [NSHP, DIM], BF).ap()
    table16 = nc.dram_tensor("table16", [N_TABLE, DIM], BF, addr_space="Shared").ap()

    with tile.TileContext(nc) as tc:
        with (
            tc.tile_pool(name="const", bufs=1) as constp,
            tc.tile_pool(name="xt", bufs=1) as xtp,
            tc.tile_pool(name="edges", bufs=1) as edgep,
            tc.tile_pool(name="dl", bufs=2) as dlp,
            tc.tile_pool(name="msg", bufs=msg_bufs) as msgp,
            tc.tile_pool(name="sel", bufs=sel_bufs) as selp,
            tc.tile_pool(name="hsb", bufs=1) as hsbp,
            tc.tile_pool(name="tr", bufs=3) as trp,
            tc.tile_pool(name="ph", bufs=ph_bufs, space="PSUM") as php,
            tc.tile_pool(name="pw", bufs=pw_bufs, space="PSUM") as pwp,
            tc.tile_pool(name="pt", bufs=pt_bufs, space="PSUM") as ptp,
            tc.tile_pool(name="pb", bufs=1, space="PSUM") as pbp,
        ):
            nc.gpsimd.load_library(library_config.mlp)

            iota_sb = constp.tile([P, P], BF)
            nc.sync.dma_start(out=iota_sb[:], in_=iota_in[:])
            identb_sb = constp.tile([P, P], BF)
            nc.sync.dma_start(out=identb_sb[:], in_=identb_in[:])
            w_sb = constp.tile([P, N_LAYERS * DIM], F32)
            for l in range(N_LAYERS):
                nc.sync.dma_start(out=w_sb[:, l * DIM : (l + 1) * DIM], in_=w_in[l])
            brow_sb = constp.tile([1, N_LAYERS * DIM], F32)
            nc.sync.dma_start(out=brow_sb[:], in_=b_in[:])
            bcol_sb = constp.tile([DIM, N_LAYERS], F32)
            nc.sync.dma_start(out=bcol_sb[:], in_=bcol_in[:])
            ones_row = constp.tile([1, P], F32)
            nc.vector.memset(ones_row[:], 1.0)

            xt_sb = xtp.tile([P, NSHP], F32)
            nc.sync.dma_start(out=xt_sb[:], in_=xt_in[:])

            for l in range(N_LAYERS):
                tl = int(tl_layer[l])
                tlc = [int(t) for t in tlc_layer[l]]
                coff = [0] * NSEG
                for s in range(1, NSEG):
                    coff[s] = coff[s - 1] + tlc[s - 1]

                # ---- normalization: dis = 1/sqrt(deg), [dest_p, w] ---------
                deg_sb = trp.tile([P, NWIN], F32, tag="deg")
                nc.sync.dma_start(out=deg_sb[:], in_=deg_in[l])
                dis_sb = trp.tile([P, NWIN], F32, tag="dis")
                nc.vector.reciprocal(dis_sb[:], deg_sb[:])
                nc.scalar.activation(
                    dis_sb[:], dis_sb[:], mybir.ActivationFunctionType.Sqrt
                )

                if l == N_LAYERS - 1:
                    # b replicated per partition (for the untransposed output)
                    pb2 = pbp.tile([P, P], F32, space="PSUM", tag="pb")
                    nc.tensor.matmul(
                        out=pb2[:],
                        lhsT=ones_row[:],
                        rhs=brow_sb[:, l * DIM : (l + 1) * DIM],
                        start=True,
                        stop=True,
                    )
                    bbc_sb = trp.tile([P, P], F32, tag="bbc")
                    nc.vector.tensor_copy(out=bbc_sb[:], in_=pb2[:])

                # ---- edge metadata ----------------------------------------
                idxs_sb = edgep.tile([P, tmax * 8], I16, tag="idxs")
                nc.sync.dma_start(out=idxs_sb[:, : tl * 8], in_=idxs_in[l, :, : tl * 8])
                dloc_sb = dlp.tile([P, tmax], BF, tag="dlocs")
                nc.sync.dma_start(out=dloc_sb[:, :tl], in_=dloc_in[l, :, :tl])

                # ---- H stage: hsb = dis (.) (X @ W), AllGather ------------
                hsb = hsbp.tile([P, NWIN * P], BF, tag="hsb")
                for w in range(NWIN):
                    ph = php.tile([P, P], F32, space="PSUM", tag="ph")
                    nc.tensor.matmul(
                        out=ph[:],
                        lhsT=xt_sb[:, w * P : (w + 1) * P],
                        rhs=w_sb[:, l * DIM : (l + 1) * DIM],
                        start=True,
                        stop=True,
                    )
                    if use_act:
                        nc.scalar.activation(
                            out=hsb[:, w * P : (w + 1) * P],
                            in_=ph[:],
                            func=mybir.ActivationFunctionType.Copy,
                            scale=dis_sb[:, w : w + 1],
                        )
                    else:
                        nc.vector.tensor_scalar(
                            out=hsb[:, w * P : (w + 1) * P],
                            in0=ph[:],
                            scalar1=dis_sb[:, w : w + 1],
                            scalar2=None,
                            op0=mybir.AluOpType.mult,
                        )
                nc.sync.dma_start(
                    out=hloc16[:].rearrange("(w p) f -> p w f", p=P),
                    in_=hsb[:].rearrange("p (w f) -> p w f", f=DIM),
                )
                if skip_collective:
                    nc.sync.dma_start(out=table16[:NSHP, :], in_=hloc16[:])
                else:
                    nc.gpsimd.collective_compute(
                        "AllGather",
                        mybir.AluOpType.bypass,
                        replica_groups=[list(range(N_CORES))],
                        ins=[hloc16[:]],
                        outs=[table16[:]],
                    )

                # ---- scatter stage ----------------------------------------
                cpos = [0] * NSEG
                cur_msg = [None] * NSEG
                s_sb = None
                t = 0
                nbat = [(tlc[s] + K - 1) // K for s in range(NSEG)]
                pending = [[] for _ in range(NSEG)]
                issued = [0] * NSEG

                def issue_gather(s):
                    b = issued[s]
                    nbg = min(K, tlc[s] - b * K)
                    mt = msgp.tile([P, K * P], BF, tag=f"m{s}", name=f"mt{l}_{s}_{b}")
                    nc.gpsimd.dma_gather(
                        mt[:, : nbg * P].rearrange("p (k f) -> p k f", k=nbg),
                        table16[s * SEGR : (s + 1) * SEGR, :],
                        idxs_sb[
                            :,
                            (coff[s] + b * K) * 8 : (coff[s] + b * K) * 8 + nbg * 8,
                        ],
                        nbg * P,
                        nbg * P,
                        P,
                        single_packet=False,
                        queue_num=s,
                    )
                    pending[s].append(mt)
                    issued[s] += 1

                for s in range(NSEG):
                    while issued[s] < min(msg_bufs, nbat[s]):
                        issue_gather(s)
                for w in range(NWIN):
                    wtot = int(m_chunks[l, w].sum())
                    pw = pwp.tile([P, P], F32, space="PSUM", tag="pw")
                    nc.tensor.matmul(
                        out=pw[:],
                        lhsT=identb_sb[:],
                        rhs=hsb[:, w * P : (w + 1) * P],
                        start=True,
                        stop=(wtot == 0),
                    )
                    done = 0
                    for s in range(NSEG):
                        for _ in range(int(m_chunks[l, w, s])):
                            if t % SB == 0:
                                nb = min(SB, tl - t)
                                s_sb = selp.tile([P, SB * P], SDT, tag="sel")
                                nc.vector.tensor_tensor(
                                    out=s_sb[:, : nb * P].rearrange(
                                        "p (k j) -> p k j", k=nb
                                    ),
                                    in0=dloc_sb[:, t : t + nb]
                                    .unsqueeze(2)
                                    .to_broadcast([P, nb, P]),
                                    in1=iota_sb[:]
                                    .unsqueeze(1)
                                    .to_broadcast([P, nb, P]),
                                    op=mybir.AluOpType.is_equal,
                                )
                            cp = cpos[s]
                            if cp % K == 0:
                                cur_msg[s] = pending[s].pop(0)
                                if issued[s] < nbat[s]:
                                    issue_gather(s)
                            done += 1
                            nc.tensor.matmul(
                                out=pw[:],
                                lhsT=s_sb[:, (t % SB) * P : (t % SB + 1) * P],
                                rhs=cur_msg[s][:, (cp % K) * P : (cp % K + 1) * P],
                                start=False,
                                stop=(done == wtot),
                            )
                            cpos[s] += 1
                            t += 1
                    # ---- epilogue -------------------------------------
                    if l < N_LAYERS - 1:
                        # relu and +b commute with the transpose: apply
                        # dis before, b (per-partition post-transpose) after
                        t2 = trp.tile([P, P], BF, tag="t2b")
                        if use_act:
                            nc.scalar.activation(
                                out=t2[:],
                                in_=pw[:],
                                func=mybir.ActivationFunctionType.Copy,
                                scale=dis_sb[:, w : w + 1],
                            )
                        else:
                            nc.vector.tensor_scalar(
                                out=t2[:],
                                in0=pw[:],
                                scalar1=dis_sb[:, w : w + 1],
                                scalar2=None,
                                op0=mybir.AluOpType.mult,
                            )
                        pt = ptp.tile([P, P], BF, space="PSUM", tag="pt")
                        nc.tensor.transpose(
                            out=pt[:], in_=t2[:], identity=identb_sb[:]
                        )
                        if use_act:
                            nc.scalar.activation(
                                out=xt_sb[:, w * P : (w + 1) * P],
                                in_=pt[:],
                                func=mybir.ActivationFunctionType.Relu,
                                bias=bcol_sb[:, l : l + 1],
                            )
                        else:
                            nc.vector.tensor_scalar(
                                out=xt_sb[:, w * P : (w + 1) * P],
                                in0=pt[:],
                                scalar1=bcol_sb[:, l : l + 1],
                                scalar2=0.0,
                                op0=mybir.AluOpType.add,
                                op1=mybir.AluOpType.max,
                            )
                    else:
                        t1 = trp.tile([P, P], F32, tag="t1f")
                        if use_act:
                            nc.scalar.activation(
                                out=t1[:],
                                in_=pw[:],
                                func=mybir.ActivationFunctionType.Copy,
                                scale=dis_sb[:, w : w + 1],
                            )
                        else:
                            nc.vector.tensor_scalar(
                                out=t1[:],
                                in0=pw[:],
                                scalar1=dis_sb[:, w : w + 1],
                                scalar2=None,
                                op0=mybir.AluOpType.mult,
                            )
                        t2f = trp.tile([P, P], F32, tag="t2f")
                        nc.vector.tensor_tensor(
                            out=t2f[:],
                            in0=t1[:],
                            in1=bbc_sb[:],
                            op=mybir.AluOpType.add,
                        )
                        nc.vector.tensor_scalar(
                            out=t2f[:],
                            in0=t2f[:],
                            scalar1=0.0,
                            scalar2=None,
                            op0=mybir.AluOpType.max,
                        )
                        nc.sync.dma_start(
                            out=out_ap[w * P : (w + 1) * P, :], in_=t2f[:]
                        )

    nc.compile()
    return nc


def build_all(x, edge_indices, W, b):
    in_maps, layout = _prepare(x, edge_indices, W, b)
    nc = _build(layout)
    return nc, in_maps


def kernel(x, edge_indices, W, b):
    nc, in_maps = build_all(x, edge_indices, W, b)
    last_err = None
    for _ in range(3):  # retry transient NRT/axon device faults
        try:
            res = run_bass_kernel_spmd(nc, in_maps, list(range(N_CORES)))
            break
        except Exception as e:  # noqa: BLE001
            last_err = e
            import time as _time

            _time.sleep(5.0)
    else:
        raise last_err
    out = np.concatenate(
        [res.results[c]["out"][:NSH] for c in range(N_CORES)], axis=0
    )
    return out.astype(np.float32)
